# revision 27
# baseline (speedup 1.0000x reference)
"""Trainium2 Bass kernel for BCNLayer (3x3 per-position-weighted spatial
shift conv over a 128x128 grid + sigmoid).

y[yo,xo,b] = sigmoid( sum_{dy,dx in {-1,0,1}} w[dy+1,dx+1,(yo-dy)*128+(xo-dx)]
                      * x[(yo-dy)*128+(xo-dx), b] )   (zero outside the grid)

Formulation: for each output row yo, y_row[yo] = sigmoid( sum_{yi in
{yo-1,yo,yo+1}} T[dy,yi].T @ x_row[yi] ) where T[dy,yi] is a 128x128
tridiagonal matrix holding the three dx weight vectors of input row yi on
its diagonals (dy = yo-yi).  T matrices are built on-chip from an SBUF
weight image with one-hot diagonal masks (one tensor_tensor multiply +
two predicated copies + an f32r rounding copy per dy); a 130-wide buffer
with the matmul reading cols 1:129 makes the x-boundary masking fall out
of the padding columns.  Matmuls run in float32r (1 cycle/row vs 4 for
fp32), which the TensorEngine requires of both operands, so x chunks are
loaded through SWDGE DMAs that cast f32 -> f32r inline.

Sharding: data-parallel over batch, 4096/8 = 512 columns per core.
"""

import numpy as np

H = 128
W = 128
HW = H * W
B = 4096
NCORES = 8
BC = B // NCORES  # 512 batch columns per core
G = 8  # yi rows per weight-group tile
R = 4  # x/y rows per DMA chunk (4 * 128 part * 512 * 4B = 1 MiB)

_CACHE = {}


def _make_tile_context_cls():
    import concourse.tile as tile
    import bass_rust

    class SplitDrainTileContext(tile.TileContext):
        """The walrus build in this container accepts at most one sem-wait
        per instruction; Tile freely emits several (e.g. a matmul waiting
        on both operand DMA lanes).  Split the extras onto single-wait
        nops emitted just before the instruction on the same engine."""

        def _add_instruction(self, inst):
            from concourse import mybir as _mybir

            si = inst.sync_info
            if si is not None and si.on_wait and len(si.on_wait) > 1:
                waits = list(si.on_wait)
                si.on_wait = [waits[-1]]
                for w in waits[:-1]:
                    nop = _mybir.InstNoOp(
                        name=self.nc.get_next_instruction_name(),
                        ins=[],
                        outs=[],
                    )
                    nop.engine = inst.engine
                    nop.sync_info = _mybir.SyncInfo(on_wait=[w], on_update=[])
                    super()._add_instruction(nop)
            super()._add_instruction(inst)

        def _drain_and_barrier(self, tick_clock, wait_clock):
            collector = self.nc.sync.nop(nofuse=True, hint="tail_waits")
            wait_clock.add_sem_waits(
                collector.ins,
                bass_rust.ScopedClock({None: tick_clock.global_clock}),
            )
            si = collector.ins.sync_info
            waits = list(si.on_wait) if si is not None and si.on_wait else []
            if len(waits) > 1:
                si.on_wait = [waits[0]]
                from concourse import mybir as _mybir

                for w in waits[1:]:
                    n = self.nc.sync.nop(nofuse=True, hint="tail_waits")
                    n.ins.sync_info = _mybir.SyncInfo(on_wait=[w], on_update=[])
            self.nc.sync.drain()
            self.nc.all_engine_barrier()
            assert self.sems is not None
            popped = self.nc._tile_sem_poison_stack.pop()
            assert popped is self._sem_poison
            self.nc.clear_and_free_semaphores(
                list(self.sems.allocated().values())
            )
            self.nc.all_engine_barrier()

    return SplitDrainTileContext


def _build_nc(repeat=1):
    import concourse.bass as bass
    import concourse.tile as tile
    import concourse.mybir as mybir
    from concourse.ap import AP

    tile_context_cls = _make_tile_context_cls()
    f32 = mybir.dt.float32
    nc = bass.Bass("TRN2", target_bir_lowering=False, debug=False)
    x = nc.dram_tensor("x", [HW, BC], f32, kind="ExternalInput")
    # wsb[xi, (i, yi, j)] = w[i, j, yi*128+xi]: the per-partition SBUF
    # image of the weights, prepared host-side
    wsb_d = nc.dram_tensor("wsb", [128, 1152], f32, kind="ExternalInput")
    # y stored fp16 (sigmoid output in [0,1]: adds <= ~2.4e-4 abs error)
    # and upcast to f32 on the host -- halves the output DMA traffic
    f16 = mybir.dt.float16
    y = nc.dram_tensor("y", [HW, BC], f16, kind="ExternalOutput")

    f32r = mybir.dt.float32r
    NCH = H // R  # 32 row-chunks
    NGR = H // G  # 16 weight groups
    TW = 130  # T used width: col c = xi + j, lhsT reads cols 1:129
    TWS = 131  # T stored stride (!=TW so strided APs never dim-merge)

    with tile_context_cls(nc) as tc:
        with (
            tc.tile_pool(name="cn", bufs=1) as cpool,
            tc.tile_pool(name="xp", bufs=8) as xpool,
            tc.tile_pool(name="rp", bufs=2) as rpool,
            tc.tile_pool(name="tp", bufs=5) as tpool,
            tc.tile_pool(name="op", bufs=4) as opool,
            tc.tile_pool(name="ps", bufs=8, space="PSUM") as ppool,
        ):
            # one-time: weight image, one-hot diagonal masks, zero row
            wsb = cpool.tile([128, 1152], f32)
            nc.sync.dma_start(out=wsb[:], in_=wsb_d.ap())
            i16 = mybir.dt.int16
            ones = cpool.tile([128, TW], i16)
            nc.gpsimd.memset(ones[:], 1)
            onesf = cpool.tile([128, TW], f32)
            nc.gpsimd.memset(onesf[:], 1.0)
            masks = cpool.tile([128, 3, TW], i16)
            for j in range(1, 3):
                # D_j[xi, c] = 1 where c - xi - j == 0
                nc.gpsimd.affine_select(
                    masks[:, j, :], ones[:],
                    pattern=[[1, TW]], base=-j, channel_multiplier=-1,
                    compare_op=mybir.AluOpType.is_equal, fill=0,
                )
            # f32 one-hot for j=0 (used multiplicatively: zero-fills too)
            mask0f = cpool.tile([128, TW], f32)
            nc.gpsimd.affine_select(
                mask0f[:], onesf[:],
                pattern=[[1, TW]], base=0, channel_multiplier=-1,
                compare_op=mybir.AluOpType.is_equal, fill=0.0,
            )

            xt = {}
            tt = {}

            def load_chunk(c):  # noqa: closure rebound per repeat
                if c in xt or c >= NCH:
                    return
                t = xpool.tile([128, R, BC], f32r, tag="xchunk")
                # x rows [c*R*128, (c+1)*R*128), flat row r = (c*R+cc)*128+p
                # SWDGE (gpsimd) DMA casts f32 -> f32r inline, which the
                # 1-cycle/row f32r matmul path requires of its producers
                src = AP(
                    x.ap().tensor,
                    c * R * 128 * BC,
                    [[BC, 128], [128 * BC, R], [1, BC]],
                )
                nc.gpsimd.dma_start(out=t[:], in_=src)
                xt[c] = t

            def load_group(g):
                if g in tt or g >= NGR:
                    return
                traw = rpool.tile([128, 3, G, TWS], f32, tag="Traw")
                t = tpool.tile([128, 3, G, TWS], f32r, tag="T")
                ta = traw[:]
                wv = wsb[:]
                for i in range(3):
                    out_i = AP(ta.tensor, ta.offset + i * G * TWS,
                               [[3 * G * TWS, 128], [TWS, G], [1, TW]])

                    def wb(j):
                        return AP(wv.tensor,
                                  wv.offset + i * 384 + g * G * 3 + j,
                                  [[1152, 128], [3, G], [0, TW]])

                    # j=0 as a multiply by the f32 one-hot: zero-fills the
                    # whole block and places the j=0 diagonal in one pass
                    # (on gpsimd to keep DVE free for the predicated passes)
                    m0 = mask0f[:]
                    m0b = AP(m0.tensor, m0.offset,
                             [[TW, 128], [0, G], [1, TW]])
                    nc.gpsimd.tensor_tensor(
                        out_i, m0b, wb(0), mybir.AluOpType.mult
                    )
                    for j in range(1, 3):
                        ma = masks[:, j, :]
                        mb = AP(ma.tensor, ma.offset,
                                [[3 * TW, 128], [0, G], [1, TW]])
                        nc.vector.copy_predicated(out_i, mb, wb(j))
                    # round to f32r (CopyPredicated cannot write f32r)
                    ti = t[:]
                    rnd_i = AP(ti.tensor, ti.offset + i * G * TWS,
                               [[3 * G * TWS, 128], [TWS, G], [1, TW]])
                    nc.vector.tensor_copy(rnd_i, out_i)
                tt[g] = t

            rep_range = range(repeat)
            for _rep in rep_range:
              if _rep:
                xt.clear()
                tt.clear()
              # prime the pipeline: fill every buffer slot so the DMA
              # engines saturate from t=0 instead of ramping with the loop
              for _c in range(8):
                  load_chunk(_c)
              for _g in range(5):
                  load_group(_g)

              ystage = None
              for yo in range(H):
                  # prefetch beyond what this row touches
                  load_chunk((yo + 1) // R + 1)
                  if yo % 2 == 0:
                      load_chunk((yo + 1) // R + 2)
                  load_group((yo + 1) // G + 1)
                  load_group((yo + 1) // G + 2)

                  pt = ppool.tile([128, BC], f32, tag="psum")
                  yis = [yi for yi in (yo - 1, yo, yo + 1) if 0 <= yi < H]
                  for k, yi in enumerate(yis):
                      i_dy = yo - yi + 1
                      lhsT = tt[yi // G][:, i_dy, yi % G, 1 : 1 + 128]  # [128, 128]
                      rhs = xt[yi // R][:, yi % R, :]
                      nc.tensor.matmul(
                          pt[:],
                          lhsT,
                          rhs,
                          start=(k == 0),
                          stop=(k == len(yis) - 1),
                      )

                  if yo % R == 0:
                      ystage = opool.tile([128, R, BC], f16, tag="yst")
                  nc.scalar.activation(
                      ystage[:, yo % R, :],
                      pt[:],
                      mybir.ActivationFunctionType.Sigmoid,
                  )
                  if yo % R == R - 1:
                      c = yo // R
                      dst = AP(
                          y.ap().tensor,
                          c * R * 128 * BC,
                          [[BC, 128], [128 * BC, R], [1, BC]],
                      )
                      # stores on the ACT-issued HWDGE ring so they never
                      # head-of-line-block the loads on the SP ring
                      nc.scalar.dma_start(out=dst, in_=ystage[:])
    return nc


def get_nc():
    if "nc" not in _CACHE:
        _CACHE["nc"] = _build_nc()
    return _CACHE["nc"]


def kernel(x: np.ndarray, w: np.ndarray) -> np.ndarray:
    import time as _time

    from concourse.bass_utils import run_bass_kernel_spmd

    x = np.ascontiguousarray(x, dtype=np.float32)
    wsb = np.ascontiguousarray(
        np.asarray(w, dtype=np.float32)
        .reshape(3, 3, H, W)
        .transpose(3, 0, 2, 1)
        .reshape(128, 1152)
    )
    nc = get_nc()
    in_maps = [
        {"x": x[:, i * BC : (i + 1) * BC], "wsb": wsb} for i in range(NCORES)
    ]
    # The compile hook / remote execution path occasionally fails
    # transiently (observed: a flaky walrus invocation and a recoverable
    # NRT exec error); retry a few times before giving up.
    last_exc = None
    for attempt in range(4):
        try:
            res = run_bass_kernel_spmd(
                nc, in_maps, list(range(NCORES))
            ).results
            break
        except Exception as exc:  # noqa: BLE001
            last_exc = exc
            _time.sleep(2.0 * (attempt + 1))
    else:
        raise last_exc
    return np.ascontiguousarray(
        np.concatenate([res[i]["y"] for i in range(NCORES)], axis=1),
        dtype=np.float32,
    )

